# revision 4
# baseline (speedup 1.0000x reference)
"""v6: bulk SWDGE dma_gather for the gather-bound BilinearDecoder.

scores[e] = sum_j (z[src_e] @ W)[j] * z[dst_e][j] + bias, 1M edges,
8 cores, pure edge-data-parallel.

The v5 baseline used per-partition indirect DMA: one instruction per 128
gathered rows at ~1.4 us SWDGE issue each (994 ns fixed + 0.34/desc)
-> 1.81 ms. InstDMAGatherAnt amortizes the fixed cost over thousands of
rows (one descriptor per row at 0.34 ns). Its indices are int16, so the
host buckets each core's 125k edges by (src//25k, dst//25k) into 16
groups; within a bucket both gathers use segment-local indices into a
25k-row window of z. Buckets are padded (idx 0, valid) to a uniform
per-bucket capacity across cores so the NEFF is SPMD-identical; pad
slots produce garbage scores the host drops via an edge-id map.

Compute per bucket tile [128, ncol, 64]: transpose 128x128 free-blocks
(2 edge slots) on PE, matmul by block-diag(W,W), elementwise mult by
the dst tile, free-axis reduce -> scores, exactly as v5.
"""

import numpy as np

import concourse.mybir as mybir
from concourse import bacc
from concourse.bass_utils import run_bass_kernel_spmd
from concourse.masks import make_identity
from concourse.tile import TileContext

N_CORES = 8
N_NODES = 100000
DIM = 64
N_EDGES = 1000000
N_SEG = 4
SEG = 25000
E_PER_CORE = N_EDGES // N_CORES
CAP_ALIGN = 256

F32 = mybir.dt.float32
I16 = mybir.dt.int16

_CACHE = {}


def build_bass(caps):
    """caps: tuple of 16 per-bucket slot capacities (each % 256 == 0)."""
    s_tot = int(sum(caps))
    nc = bacc.Bacc()
    z_d = nc.declare_dram_parameter("z", [N_NODES, DIM], F32, isOutput=False)
    w_d = nc.declare_dram_parameter("W", [DIM, DIM], F32, isOutput=False)
    bias_d = nc.declare_dram_parameter("biasb", [128, 1], F32, isOutput=False)
    srci_d = nc.declare_dram_parameter("srci", [128, s_tot // 16], I16, isOutput=False)
    dsti_d = nc.declare_dram_parameter("dsti", [128, s_tot // 16], I16, isOutput=False)
    out_d = nc.declare_dram_parameter("out", [s_tot], F32, isOutput=True)

    with TileContext(nc) as tc:
        with (
            tc.tile_pool(name="const", bufs=1) as cpool,
            tc.tile_pool(name="gather", bufs=2) as gpool,
            tc.tile_pool(name="work", bufs=3) as wpool,
            tc.tile_pool(name="ps", bufs=3, space="PSUM") as ppool,
        ):
            ident = cpool.tile([128, 128], F32)
            make_identity(nc, ident[:])
            wbd = cpool.tile([128, 128], F32)
            nc.vector.memset(wbd[:], 0.0)
            nc.sync.dma_start(out=wbd[0:64, 0:64], in_=w_d[:, :])
            nc.sync.dma_start(out=wbd[64:128, 64:128], in_=w_d[:, :])
            bias_t = cpool.tile([128, 1], F32)
            nc.sync.dma_start(out=bias_t[:], in_=bias_d[:, :])
            srci_t = cpool.tile([128, s_tot // 16], I16)
            nc.sync.dma_start(out=srci_t[:], in_=srci_d[:, :])
            dsti_t = cpool.tile([128, s_tot // 16], I16)
            nc.sync.dma_start(out=dsti_t[:], in_=dsti_d[:, :])

            # single_packet=True overflows a packet limit past ~512 idxs;
            # with False one gather instruction handles a whole bucket.
            CHUNK = 8960
            off = 0
            for b in range(16):
                cap = int(caps[b])
                seg_s, seg_d = b // N_SEG, b % N_SEG
                coff = 0
                while coff < cap:
                    n = min(CHUNK, cap - coff)
                    ncol = n // 128
                    lo = off + coff
                    a_t = gpool.tile([128, ncol * DIM], F32, tag="A")
                    b_t = gpool.tile([128, ncol * DIM], F32, tag="B")
                    nc.gpsimd.dma_gather(
                        a_t[:].rearrange("p (k d) -> p k d", d=DIM),
                        z_d[seg_s * SEG:(seg_s + 1) * SEG, :],
                        srci_t[:, lo // 16:(lo + n) // 16],
                        n,
                        n,
                        DIM,
                        single_packet=False,
                    )
                    nc.gpsimd.dma_gather(
                        b_t[:].rearrange("p (k d) -> p k d", d=DIM),
                        z_d[seg_d * SEG:(seg_d + 1) * SEG, :],
                        dsti_t[:, lo // 16:(lo + n) // 16],
                        n,
                        n,
                        DIM,
                        single_packet=False,
                    )
                    scores = wpool.tile([128, ncol], F32, tag="scores")
                    for g in range(ncol // 2):
                        fs = slice(g * 128, (g + 1) * 128)
                        tp = ppool.tile([128, 128], F32, tag="tp")
                        nc.tensor.transpose(
                            out=tp[:], in_=a_t[:, fs], identity=ident[:]
                        )
                        at = wpool.tile([128, 128], F32, tag="at")
                        nc.scalar.copy(out=at[:], in_=tp[:])
                        cp = ppool.tile([128, 128], F32, tag="cp")
                        nc.tensor.matmul(
                            out=cp[:], lhsT=at[:], rhs=wbd[:], start=True, stop=True
                        )
                        prod = wpool.tile([128, 128], F32, tag="prod")
                        nc.vector.tensor_tensor(
                            out=prod[:], in0=cp[:], in1=b_t[:, fs],
                            op=mybir.AluOpType.mult,
                        )
                        nc.vector.reduce_sum(
                            out=scores[:, g * 2:(g + 1) * 2],
                            in_=prod[:].rearrange("p (s d) -> p s d", d=DIM),
                            axis=mybir.AxisListType.X,
                        )
                    nc.vector.tensor_scalar_add(
                        out=scores[:], in0=scores[:], scalar1=bias_t[:, :1]
                    )
                    # slot j = k*128 + p holds score[p, k]
                    nc.sync.dma_start(
                        out=out_d[lo:lo + n].rearrange("(k p) -> p k", p=128),
                        in_=scores[:],
                    )
                    coff += n
                off += cap
    nc.compile()
    return nc


def _round_up(x, m):
    return -(-x // m) * m


def _make_plan(src, dst):
    """Bucket each core's edge block by (src seg, dst seg).

    Returns (caps, s_tot, srci, dsti, eids): caps tuple[16], srci/dsti
    [N_CORES, 128, s_tot//16] int16 wrapped idx layout, eids
    [N_CORES, s_tot] int64 (-1 = pad).
    """
    bid = (src // SEG) * N_SEG + (dst // SEG)
    counts = np.zeros((N_CORES, 16), np.int64)
    orders = []
    for c in range(N_CORES):
        sl = slice(c * E_PER_CORE, (c + 1) * E_PER_CORE)
        b = bid[sl]
        counts[c] = np.bincount(b, minlength=16)
        orders.append(np.argsort(b, kind="stable") + sl.start)
    caps = tuple(int(_round_up(m, CAP_ALIGN)) for m in counts.max(axis=0))
    s_tot = int(sum(caps))
    offs = np.zeros(17, np.int64)
    np.cumsum(caps, out=offs[1:])
    srcl = np.zeros((N_CORES, s_tot), np.int16)
    dstl = np.zeros((N_CORES, s_tot), np.int16)
    eids = np.full((N_CORES, s_tot), -1, np.int64)
    for c in range(N_CORES):
        order = orders[c]
        # bucket b's edges occupy order[start:start+cnt], slot off_b..+cnt
        start = 0
        for b in range(16):
            cnt = int(counts[c, b])
            e = order[start:start + cnt]
            pos = offs[b] + np.arange(cnt)
            srcl[c, pos] = (src[e] - (b // N_SEG) * SEG).astype(np.int16)
            dstl[c, pos] = (dst[e] - (b % N_SEG) * SEG).astype(np.int16)
            eids[c, pos] = e
            start += cnt
    # wrap: idx i -> [i % 16, i // 16], replicated to 128 partitions
    srci = np.tile(srcl.reshape(N_CORES, s_tot // 16, 16).transpose(0, 2, 1), (1, 8, 1))
    dsti = np.tile(dstl.reshape(N_CORES, s_tot // 16, 16).transpose(0, 2, 1), (1, 8, 1))
    return caps, s_tot, np.ascontiguousarray(srci), np.ascontiguousarray(dsti), eids


def _run(z, edge_index, W, bias, trace):
    z = np.ascontiguousarray(np.asarray(z, dtype=np.float32))
    W = np.ascontiguousarray(np.asarray(W, dtype=np.float32))
    bias_f = np.float32(np.asarray(bias).reshape(-1)[0])
    ei = np.asarray(edge_index)
    src = ei[0].astype(np.int64)
    dst = ei[1].astype(np.int64)
    caps, s_tot, srci, dsti, eids = _make_plan(src, dst)
    if ("nc", caps) not in _CACHE:
        _CACHE[("nc", caps)] = build_bass(caps)
    nc = _CACHE[("nc", caps)]
    biasb = np.full((128, 1), bias_f, dtype=np.float32)
    in_maps = [
        {"z": z, "W": W, "biasb": biasb, "srci": srci[c], "dsti": dsti[c]}
        for c in range(N_CORES)
    ]
    res = run_bass_kernel_spmd(nc, in_maps, list(range(N_CORES)), trace=trace)
    out = np.empty(N_EDGES, np.float32)
    for c in range(N_CORES):
        sc = np.asarray(res.results[c]["out"]).reshape(-1)
        m = eids[c] >= 0
        out[eids[c][m]] = sc[m]
    return out, res.exec_time_ns


def kernel(z, edge_index, W, bias):
    return _run(z, edge_index, W, bias, trace=False)[0]


def kernel_traced(z, edge_index, W, bias):
    """Same but profiled; returns (out, exec_ns)."""
    return _run(z, edge_index, W, bias, trace=True)


# revision 8
# speedup vs baseline: 2.2372x; 2.2372x over previous
"""v6: bulk SWDGE dma_gather for the gather-bound BilinearDecoder.

scores[e] = sum_j (z[src_e] @ W)[j] * z[dst_e][j] + bias, 1M edges,
8 cores, pure edge-data-parallel.

The v5 baseline used per-partition indirect DMA: one instruction per 128
gathered rows at ~1.4 us SWDGE issue each (994 ns fixed + 0.34/desc)
-> 1.81 ms. InstDMAGatherAnt amortizes the fixed cost over thousands of
rows (one descriptor per row at 0.34 ns). Its indices are int16, so the
host buckets each core's 125k edges by (src//25k, dst//25k) into 16
groups; within a bucket both gathers use segment-local indices into a
25k-row window of z. Buckets are padded (idx 0, valid) to a uniform
per-bucket capacity across cores so the NEFF is SPMD-identical; pad
slots produce garbage scores the host drops via an edge-id map.

Compute per bucket tile [128, ncol, 64]: transpose 128x128 free-blocks
(2 edge slots) on PE, matmul by block-diag(W,W), elementwise mult by
the dst tile, free-axis reduce -> scores, exactly as v5.
"""

import numpy as np

import concourse.mybir as mybir
import concourse.tile_sem_assignment as _tsa
from concourse import bacc, bass_isa
from concourse.bass_utils import run_bass_kernel_spmd
from concourse.masks import make_identity
from concourse.tile import TileContext

# Tile's sem pass round-robins Pool-engine DMAs over the 8 DMASW lanes with
# no regard for the SWDGE queue, but each DMASW semaphore is locked to one
# queue by the ucode. Make the lane choice queue-aware: queue q owns lanes
# {2q, 2q+1}.
_orig_assign_tick = _tsa.TileClockTick._assign_tick


def _queue_aware_assign_tick(self, inst):
    if (
        isinstance(inst, _tsa.DMAInst)
        and inst.engine == mybir.EngineType.Pool
        and not isinstance(inst, bass_isa.UserSyncedRemoteDMADescs)
    ):
        q = getattr(inst, "queue_num", 0) or 0
        cnts = self.__dict__.setdefault("_q_lane_counts", {})
        c = cnts.get(q, 0)
        cnts[q] = c + 1
        self.next_sw_dma_idx = (q * 2 + c % 2) % 8
    return _orig_assign_tick(self, inst)


_tsa.TileClockTick._assign_tick = _queue_aware_assign_tick

N_CORES = 8
N_NODES = 100000
DIM = 64
N_EDGES = 1000000
N_SEG = 4
SEG = 25000
E_PER_CORE = N_EDGES // N_CORES
CAP_ALIGN = 256

F32 = mybir.dt.float32
I16 = mybir.dt.int16

_CACHE = {}


def build_bass(caps):
    """caps: tuple of 16 per-bucket slot capacities (each % 256 == 0)."""
    s_tot = int(sum(caps))
    nc = bacc.Bacc(num_swdge_queues=4)
    z_d = nc.declare_dram_parameter("z", [N_NODES, DIM], F32, isOutput=False)
    w_d = nc.declare_dram_parameter("W", [DIM, DIM], F32, isOutput=False)
    bias_d = nc.declare_dram_parameter("biasb", [128, 1], F32, isOutput=False)
    srci_d = nc.declare_dram_parameter("srci", [128, s_tot // 16], I16, isOutput=False)
    dsti_d = nc.declare_dram_parameter("dsti", [128, s_tot // 16], I16, isOutput=False)
    out_d = nc.declare_dram_parameter("out", [s_tot], F32, isOutput=True)

    with TileContext(nc) as tc:
        with (
            tc.tile_pool(name="const", bufs=1) as cpool,
            tc.tile_pool(name="gather", bufs=2) as gpool,
            tc.tile_pool(name="work", bufs=3) as wpool,
            tc.tile_pool(name="ps", bufs=3, space="PSUM") as ppool,
        ):
            ident = cpool.tile([128, 128], F32)
            make_identity(nc, ident[:])
            wbd = cpool.tile([128, 128], F32)
            nc.vector.memset(wbd[:], 0.0)
            nc.sync.dma_start(out=wbd[0:64, 0:64], in_=w_d[:, :])
            nc.sync.dma_start(out=wbd[64:128, 64:128], in_=w_d[:, :])
            bias_t = cpool.tile([128, 1], F32)
            nc.sync.dma_start(out=bias_t[:], in_=bias_d[:, :])
            srci_t = cpool.tile([128, s_tot // 16], I16)
            nc.sync.dma_start(out=srci_t[:], in_=srci_d[:, :])
            dsti_t = cpool.tile([128, s_tot // 16], I16)
            nc.sync.dma_start(out=dsti_t[:], in_=dsti_d[:, :])

            # single_packet=True overflows a packet limit past ~512 idxs;
            # with False one gather instruction handles a whole bucket.
            # Gathers round-robin the 4 SWDGE queues (parallel Q7 desc-gen).
            CHUNK = 8960
            off = 0
            qn = 0
            for b in range(16):
                cap = int(caps[b])
                seg_s, seg_d = b // N_SEG, b % N_SEG
                coff = 0
                while coff < cap:
                    n = min(CHUNK, cap - coff)
                    ncol = n // 128
                    lo = off + coff
                    a_t = gpool.tile([128, ncol * DIM], F32, tag="A")
                    b_t = gpool.tile([128, ncol * DIM], F32, tag="B")
                    nc.gpsimd.dma_gather(
                        a_t[:].rearrange("p (k d) -> p k d", d=DIM),
                        z_d[seg_s * SEG:(seg_s + 1) * SEG, :],
                        srci_t[:, lo // 16:(lo + n) // 16],
                        n,
                        n,
                        DIM,
                        single_packet=False,
                        queue_num=qn % 4,
                    )
                    nc.gpsimd.dma_gather(
                        b_t[:].rearrange("p (k d) -> p k d", d=DIM),
                        z_d[seg_d * SEG:(seg_d + 1) * SEG, :],
                        dsti_t[:, lo // 16:(lo + n) // 16],
                        n,
                        n,
                        DIM,
                        single_packet=False,
                        queue_num=(qn + 1) % 4,
                    )
                    qn += 2
                    # zw collects W-transformed src rows for the whole chunk.
                    # Groups of 4 transposes/matmuls share one PSUM bank so
                    # PSUM->SBUF copies are 512 wide; mult runs in-place on zw
                    # and reduce covers the chunk in one DVE op each.
                    zw = wpool.tile([128, ncol * DIM], F32, tag="zw")
                    ngrp = ncol // 2
                    for g0 in range(0, ngrp, 4):
                        gw = min(4, ngrp - g0)
                        tp = ppool.tile([128, gw * 128], F32, tag="tp")
                        for i in range(gw):
                            nc.tensor.transpose(
                                out=tp[:, i * 128:(i + 1) * 128],
                                in_=a_t[:, (g0 + i) * 128:(g0 + i + 1) * 128],
                                identity=ident[:],
                            )
                        at = wpool.tile([128, gw * 128], F32, tag="at")
                        nc.scalar.copy(out=at[:], in_=tp[:])
                        cp = ppool.tile([128, gw * 128], F32, tag="cp")
                        for i in range(gw):
                            nc.tensor.matmul(
                                out=cp[:, i * 128:(i + 1) * 128],
                                lhsT=at[:, i * 128:(i + 1) * 128],
                                rhs=wbd[:],
                                start=True,
                                stop=True,
                            )
                        nc.scalar.copy(
                            out=zw[:, g0 * 128:(g0 + gw) * 128], in_=cp[:]
                        )
                    nc.vector.tensor_tensor(
                        out=zw[:], in0=zw[:], in1=b_t[:],
                        op=mybir.AluOpType.mult,
                    )
                    scores = wpool.tile([128, ncol], F32, tag="scores")
                    nc.vector.reduce_sum(
                        out=scores[:],
                        in_=zw[:].rearrange("p (s d) -> p s d", d=DIM),
                        axis=mybir.AxisListType.X,
                    )
                    nc.vector.tensor_scalar_add(
                        out=scores[:], in0=scores[:], scalar1=bias_t[:, :1]
                    )
                    # slot j = k*128 + p holds score[p, k]
                    nc.sync.dma_start(
                        out=out_d[lo:lo + n].rearrange("(k p) -> p k", p=128),
                        in_=scores[:],
                    )
                    coff += n
                off += cap
    nc.compile()
    return nc


def _round_up(x, m):
    return -(-x // m) * m


def _make_plan(src, dst):
    """Bucket each core's edge block by (src seg, dst seg).

    Returns (caps, s_tot, srci, dsti, eids): caps tuple[16], srci/dsti
    [N_CORES, 128, s_tot//16] int16 wrapped idx layout, eids
    [N_CORES, s_tot] int64 (-1 = pad).
    """
    bid = (src // SEG) * N_SEG + (dst // SEG)
    counts = np.zeros((N_CORES, 16), np.int64)
    orders = []
    for c in range(N_CORES):
        sl = slice(c * E_PER_CORE, (c + 1) * E_PER_CORE)
        b = bid[sl]
        counts[c] = np.bincount(b, minlength=16)
        orders.append(np.argsort(b, kind="stable") + sl.start)
    caps = tuple(int(_round_up(m, CAP_ALIGN)) for m in counts.max(axis=0))
    s_tot = int(sum(caps))
    offs = np.zeros(17, np.int64)
    np.cumsum(caps, out=offs[1:])
    srcl = np.zeros((N_CORES, s_tot), np.int16)
    dstl = np.zeros((N_CORES, s_tot), np.int16)
    eids = np.full((N_CORES, s_tot), -1, np.int64)
    for c in range(N_CORES):
        order = orders[c]
        # bucket b's edges occupy order[start:start+cnt], slot off_b..+cnt
        start = 0
        for b in range(16):
            cnt = int(counts[c, b])
            e = order[start:start + cnt]
            pos = offs[b] + np.arange(cnt)
            srcl[c, pos] = (src[e] - (b // N_SEG) * SEG).astype(np.int16)
            dstl[c, pos] = (dst[e] - (b % N_SEG) * SEG).astype(np.int16)
            eids[c, pos] = e
            start += cnt
    # wrap: idx i -> [i % 16, i // 16], replicated to 128 partitions
    srci = np.tile(srcl.reshape(N_CORES, s_tot // 16, 16).transpose(0, 2, 1), (1, 8, 1))
    dsti = np.tile(dstl.reshape(N_CORES, s_tot // 16, 16).transpose(0, 2, 1), (1, 8, 1))
    return caps, s_tot, np.ascontiguousarray(srci), np.ascontiguousarray(dsti), eids


def _run(z, edge_index, W, bias, trace):
    z = np.ascontiguousarray(np.asarray(z, dtype=np.float32))
    W = np.ascontiguousarray(np.asarray(W, dtype=np.float32))
    bias_f = np.float32(np.asarray(bias).reshape(-1)[0])
    ei = np.asarray(edge_index)
    src = ei[0].astype(np.int64)
    dst = ei[1].astype(np.int64)
    caps, s_tot, srci, dsti, eids = _make_plan(src, dst)
    if ("nc", caps) not in _CACHE:
        _CACHE[("nc", caps)] = build_bass(caps)
    nc = _CACHE[("nc", caps)]
    biasb = np.full((128, 1), bias_f, dtype=np.float32)
    in_maps = [
        {"z": z, "W": W, "biasb": biasb, "srci": srci[c], "dsti": dsti[c]}
        for c in range(N_CORES)
    ]
    res = run_bass_kernel_spmd(nc, in_maps, list(range(N_CORES)), trace=trace)
    out = np.empty(N_EDGES, np.float32)
    for c in range(N_CORES):
        sc = np.asarray(res.results[c]["out"]).reshape(-1)
        m = eids[c] >= 0
        out[eids[c][m]] = sc[m]
    return out, res.exec_time_ns


def kernel(z, edge_index, W, bias):
    return _run(z, edge_index, W, bias, trace=False)[0]


def kernel_traced(z, edge_index, W, bias):
    """Same but profiled; returns (out, exec_ns)."""
    return _run(z, edge_index, W, bias, trace=True)
